# revision 1
# baseline (speedup 1.0000x reference)
"""MoE clustered attention kernel for Trainium2 (8 NeuronCores).

Problem: B=2, LQ=LK=2048, D=1024, H=16 heads (DH=64), M=8 clusters.
Each query/key token is routed (argmax of X @ Wr) to one of 8 clusters;
attention is only computed within a cluster (block-sparse attention).

Strategy
--------
Host side:
  * compute router assignments with numpy fp32 (verified to match the
    jax reference on every argmax decision; min top-2 logit gap for
    these inputs is 1.5e-4, far above fp32/bf16 rounding noise),
  * gather tokens by cluster so in-cluster attention becomes block
    attention on contiguous ranges; pad each cluster to a common
    cross-batch geometry (queries to >=256 and even, keys to multiples
    of 128) so one SPMD program serves both batches,
  * pre-transpose X to [D, L] so on-device projections contract over
    the partition dimension directly,
  * append 9 "mask rows" to the per-head qT/kT tensors: the scores
    matmul contracts over 64+9=73 rows and the extra rows add exactly
    0 to same-cluster pairs and exactly -16384 to cross-cluster or
    padded pairs (exp(x-16384) == 0), which makes every (k-slice,
    q-slice) block correct regardless of cluster boundaries and
    handles padding for free (all mask constants are powers of two,
    exact in bf16, and cancel exactly inside the matmul).

Device side (per core; core = batch * 4 + head_group, 4 heads each):
  * per-head qT/kT projections in transposed layout [73, L],
  * v in natural layout [tokens, 4 heads x (64 + ones col)]; the ones
    column makes the ctx matmul emit the softmax denominator as row 64,
  * per (head, cluster): one K=73 scores matmul per 128-key chunk,
    exp on ScalarE (no max-subtraction needed: scores are O(30); the
    masked entries underflow to exactly 0), ctx matmul accumulation,
    denominator rows staged into a packed [32, 512] tensor,
  * one batched reciprocal over all 32 denominator rows, GpSimd
    partition-broadcast per (head, cluster), in-place normalize on
    VectorE, then the output projection (partial over 4 heads).
Host sums the 4 head-group partials per batch and un-permutes rows.

Matmul dtype is bf16 by default (fp32 PSUM accumulation; measured end
to end relative error ~4.6e-3). Set BASS_MM_DTYPE=f32r for the fp32r
variant (~3e-4, ~1.5x slower).
"""

import os

import numpy as np
import ml_dtypes

import concourse.bacc as bacc
import concourse.tile as tile
import concourse.mybir as mybir
from concourse.bass_utils import run_bass_kernel_spmd

F32 = mybir.dt.float32
F32R = mybir.dt.float32r
BF16 = mybir.dt.bfloat16
EXP = mybir.ActivationFunctionType.Exp
MULT = mybir.AluOpType.mult

H = 16            # total heads
HPC = 4           # heads per core
N_CORES = 8
SQRT_BIG = 128.0  # sqrt(16384); mask contributions are exact powers of two

MMDT = F32R if os.environ.get("BASS_MM_DTYPE") == "f32r" else BF16


def _ceil_to(x, m):
    return (x + m - 1) // m * m


def _plan(aq, ak, M):
    """Common (cross-batch) padded cluster geometry."""
    B = aq.shape[0]
    nq = np.array([[int((aq[b] == c).sum()) for c in range(M)] for b in range(B)])
    nk = np.array([[int((ak[b] == c).sum()) for c in range(M)] for b in range(B)])
    # fp32r matmuls require an even moving free dim -> round up to even
    NQP = [max(256, _ceil_to(int(nq[:, c].max()), 2)) for c in range(M)]
    NKP = [_ceil_to(max(128, int(nk[:, c].max())), 128) for c in range(M)]
    qoff = np.concatenate([[0], np.cumsum(NQP)])
    koff = np.concatenate([[0], np.cumsum(NKP)])
    LQG = _ceil_to(int(qoff[-1]), 256)
    NKG = _ceil_to(int(koff[-1]), 256)
    return NQP, NKP, qoff[:-1].tolist(), koff[:-1].tolist(), LQG, NKG


def _build_program(NQP, NKP, qoffs, koffs, LQG, NKG, D):
    nc = bacc.Bacc("TRN2", target_bir_lowering=False, debug=False)
    XQT = nc.dram_tensor("XQT", [D, LQG], MMDT, kind="ExternalInput").ap()
    XKT = nc.dram_tensor("XKT", [D, NKG], MMDT, kind="ExternalInput").ap()
    XVT = nc.dram_tensor("XVT", [D, NKG], MMDT, kind="ExternalInput").ap()
    WQ = nc.dram_tensor("WQ", [D, 256], MMDT, kind="ExternalInput").ap()
    WK = nc.dram_tensor("WK", [D, 256], MMDT, kind="ExternalInput").ap()
    WV = nc.dram_tensor("WV", [D, 256], MMDT, kind="ExternalInput").ap()
    WO = nc.dram_tensor("WO", [256, D], MMDT, kind="ExternalInput").ap()
    MQ = nc.dram_tensor("MQ", [9, LQG], MMDT, kind="ExternalInput").ap()
    MK = nc.dram_tensor("MK", [9, NKG], MMDT, kind="ExternalInput").ap()
    OUT = nc.dram_tensor("OUT", [LQG, D], F32, kind="ExternalOutput").ap()

    ND = D // 128          # contraction chunks (8)
    NVC = NKG // 128       # value token chunks
    M = len(NQP)

    def ms_view(ap):
        # memset can't write fp32r dtype; write the same bits as fp32
        return ap.bitcast(F32) if MMDT == F32R else ap

    with tile.TileContext(nc) as tc:
        with (
            tc.tile_pool(name="weights", bufs=1) as wpool,
            tc.tile_pool(name="proj_out", bufs=1) as projpool,
            tc.tile_pool(name="psA", bufs=2, space="PSUM") as psA,
            tc.tile_pool(name="psB", bufs=2, space="PSUM") as psB,
            tc.tile_pool(name="psC", bufs=2, space="PSUM") as psC,
        ):
            wq = wpool.tile([128, ND * 256], MMDT, tag="wq")
            wk = wpool.tile([128, ND * 256], MMDT, tag="wk")
            wv = wpool.tile([128, ND * 256], MMDT, tag="wv")
            wo = wpool.tile([128, 2 * 1024], MMDT, tag="wo")
            nc.sync.dma_start(wq[:].rearrange("p (n m) -> p n m", n=ND),
                              WQ.rearrange("(n p) m -> p n m", p=128))
            nc.sync.dma_start(wk[:].rearrange("p (n m) -> p n m", n=ND),
                              WK.rearrange("(n p) m -> p n m", p=128))
            nc.sync.dma_start(wv[:].rearrange("p (n m) -> p n m", n=ND),
                              WV.rearrange("(n p) m -> p n m", p=128))
            nc.sync.dma_start(wo[:].rearrange("p (n m) -> p n m", n=2),
                              WO.rearrange("(n p) m -> p n m", p=128))

            # per-head [73, L]: rows 0..63 head dims, rows 64..72 mask rows
            qT = [projpool.tile([73, LQG], MMDT, tag=f"qT{h}", name=f"qT{h}")
                  for h in range(HPC)]
            kT = [projpool.tile([73, NKG], MMDT, tag=f"kT{h}", name=f"kT{h}")
                  for h in range(HPC)]
            vA = projpool.tile([128, NVC * 260], MMDT, tag="vA")
            ctxT = [projpool.tile([128, LQG], MMDT, tag=f"ctxT{p}", name=f"ctxT{p}")
                    for p in range(2)]
            # denominator rows: head h's clusters at quad-aligned rows 32h..32h+M
            dn = projpool.tile([128, 512], F32, tag="dn")
            rcp = projpool.tile([128, 512], F32, tag="rcp")

            for h in range(HPC):
                nc.sync.dma_start(qT[h][64:73, :], MQ)
                nc.sync.dma_start(kT[h][64:73, :], MK)

            # ones columns of v_aug (col 64 of each head's 65-wide block):
            # memset everything to 1.0; the projection copies below
            # overwrite the 4x64 value columns, leaving col 64 at 1.0.
            nc.vector.memset(ms_view(vA[:]), 1.0)
            # zero the tail columns of ctxT that attention never writes
            tail = int(np.sum(NQP))
            if tail < LQG:
                for p in range(2):
                    nc.vector.memset(ms_view(ctxT[p][:, tail:LQG]), 0.0)

            # ---- projections ----
            with tc.tile_pool(name="xin", bufs=3) as xpool:
                def proj_T(xdram, L, wtile, dest):
                    """dest[h][0:64, L] = (W_h.T @ X^T), streamed over L."""
                    for off in range(0, L, 512):
                        w = min(512, L - off)
                        xt = xpool.tile([128, ND, 512], MMDT, tag="xt")
                        nc.sync.dma_start(
                            xt[:, :, :w],
                            xdram.rearrange("(n p) m -> p n m", p=128)[:, :, off:off + w])
                        for pair in range(2):
                            ps = psA.tile([128, 512], F32, tag="psproj")
                            for half in range(0, w, 256):
                                w2 = min(256, w - half)
                                for d in range(ND):
                                    nc.tensor.matmul(
                                        ps[:, half:half + w2],
                                        wtile[:, d * 256 + pair * 128: d * 256 + (pair + 1) * 128],
                                        xt[:, d, half:half + w2],
                                        start=(d == 0), stop=(d == ND - 1))
                            if pair == 0:
                                nc.vector.tensor_copy(dest[0][0:64, off:off + w], ps[0:64, :w])
                                nc.scalar.copy(dest[1][0:64, off:off + w], ps[64:128, :w])
                            else:
                                nc.scalar.copy(dest[2][0:64, off:off + w], ps[0:64, :w])
                                nc.vector.tensor_copy(dest[3][0:64, off:off + w], ps[64:128, :w])

                proj_T(XQT, LQG, wq, qT)
                proj_T(XKT, NKG, wk, kT)

                # value projection: natural layout, 4 heads + ones col
                for off in range(0, NKG, 512):
                    w = min(512, NKG - off)
                    xt = xpool.tile([128, ND, 512], MMDT, tag="xt")
                    nc.sync.dma_start(
                        xt[:, :, :w],
                        XVT.rearrange("(n p) m -> p n m", p=128)[:, :, off:off + w])
                    for sub in range(w // 128):
                        tc128 = off // 128 + sub
                        ps = psA.tile([128, 256], F32, tag="psproj")
                        for d in range(ND):
                            nc.tensor.matmul(ps[:],
                                             xt[:, d, sub * 128:(sub + 1) * 128],
                                             wv[:, d * 256:(d + 1) * 256],
                                             start=(d == 0), stop=(d == ND - 1))
                        nc.vector.tensor_copy(
                            vA[:].rearrange("p (c h e) -> p c h e", c=NVC, h=HPC)[:, tc128, :, 0:64],
                            ps[:].rearrange("p (h e) -> p h e", h=HPC))

            # ---- clustered attention ----
            with tc.tile_pool(name="epool", bufs=3) as epool, \
                 tc.tile_pool(name="btpool", bufs=4) as btpool:
                for h in range(HPC):
                    pair, rb = h // 2, (h % 2) * 64
                    for c in range(M):
                        qo, nqp = qoffs[c], NQP[c]
                        nkc = NKP[c] // 128
                        # scores into 2-bank super tiles; exp reads pairs of
                        # banks in one strided ACTIVATE (halves ACT overhead)
                        sts, es, eslice = [], [], []
                        for ki in range(0, nkc, 2):
                            nk2 = min(2, nkc - ki)
                            ps_s = psB.tile([128, 1024], F32, tag="ps_s")
                            e = epool.tile([128, 1024], MMDT, tag="e")
                            for kj in range(nk2):
                                ko = koffs[c] + (ki + kj) * 128
                                nc.tensor.matmul(
                                    ps_s[:, kj * 512: kj * 512 + nqp],
                                    kT[h][0:73, ko:ko + 128],
                                    qT[h][0:73, qo:qo + nqp],
                                    start=True, stop=True)
                                es.append(e)
                                eslice.append(slice(kj * 512, kj * 512 + nqp))
                            pv = ps_s[:].rearrange("p (b n) -> p b n", b=2)[:, 0:nk2, 0:nqp]
                            ev = e[:].rearrange("p (b n) -> p b n", b=2)[:, 0:nk2, 0:nqp]
                            nc.scalar.activation(ev, pv, EXP)
                        ps_c = psC.tile([128, 512], F32, tag="ps_c")
                        for ki in range(nkc):
                            kc128 = koffs[c] // 128 + ki
                            nc.tensor.matmul(ps_c[:65, :nqp],
                                             vA[:, kc128 * 260 + h * 65: kc128 * 260 + (h + 1) * 65],
                                             es[ki][:, eslice[ki]],
                                             start=(ki == 0), stop=(ki == nkc - 1))
                        # evacuate unnormalized ctx; stage denominator row
                        # into this head's quad of `dn` (DMA writes may target
                        # any partition; compute engines may not)
                        nc.vector.tensor_copy(ctxT[pair][rb:rb + 64, qo:qo + nqp],
                                              ps_c[0:64, :nqp])
                        stg = btpool.tile([1, 512], F32, tag="stg")
                        nc.scalar.copy(stg[:, :nqp], ps_c[64:65, :nqp])
                        nc.sync.dma_start(dn[32 * h + c: 32 * h + c + 1, :nqp],
                                          stg[:, :nqp])
                    # all of head h's denominators staged: one batched recip,
                    # then normalize in place (overlaps head h+1's attention)
                    nc.vector.reciprocal(rcp[32 * h: 32 * h + M, :],
                                         dn[32 * h: 32 * h + M, :])
                    for c in range(M):
                        qo, nqp = qoffs[c], NQP[c]
                        btsrc = btpool.tile([1, 512], F32, tag="btsrc")
                        nc.sync.dma_start(btsrc[:, :nqp],
                                          rcp[32 * h + c: 32 * h + c + 1, :nqp])
                        bt = btpool.tile([128, 512], F32, tag="bt")
                        nc.gpsimd.partition_broadcast(bt[:, :nqp], btsrc[:, :nqp])
                        nc.vector.tensor_tensor(ctxT[pair][rb:rb + 64, qo:qo + nqp],
                                                ctxT[pair][rb:rb + 64, qo:qo + nqp],
                                                bt[rb:rb + 64, :nqp], MULT)

            # ---- output projection ----
            with tc.tile_pool(name="outsb", bufs=4) as opool:
                for mi in range(LQG // 128):
                    for n2 in range(2):
                        ps_o = psB.tile([128, 512], F32, tag="ps_s")
                        for half in range(2):
                            for pair in range(2):
                                nc.tensor.matmul(
                                    ps_o[:, half * 256:(half + 1) * 256],
                                    ctxT[pair][:, mi * 128:(mi + 1) * 128],
                                    wo[:, pair * 1024 + n2 * 512 + half * 256:
                                       pair * 1024 + n2 * 512 + (half + 1) * 256],
                                    start=(pair == 0), stop=(pair == 1))
                        ob = opool.tile([128, 512], F32, tag="ob")
                        if (mi + n2) % 2:
                            nc.scalar.copy(ob[:], ps_o[:])
                        else:
                            nc.vector.tensor_copy(ob[:], ps_o[:])
                        nc.sync.dma_start(
                            OUT[mi * 128:(mi + 1) * 128, n2 * 512:(n2 + 1) * 512], ob[:])

    nc.compile()
    return nc


_CACHE = {}


def run(inputs, trace=False):
    queries = np.asarray(inputs["queries"], np.float32)
    keys = np.asarray(inputs["keys"], np.float32)
    values = np.asarray(inputs["values"], np.float32)
    Wq = np.asarray(inputs["Wq"], np.float32)
    Wk = np.asarray(inputs["Wk"], np.float32)
    Wv = np.asarray(inputs["Wv"], np.float32)
    Wo = np.asarray(inputs["Wo"], np.float32)
    Wr = np.asarray(inputs["Wr"], np.float32)

    B, LQ, D = queries.shape
    M = Wr.shape[1]
    DH = D // H
    scale = np.float32(1.0 / np.sqrt(DH))
    npdt = ml_dtypes.bfloat16 if MMDT == BF16 else np.float32

    aq = np.argmax(queries @ Wr, axis=-1)   # [B, LQ]
    ak = np.argmax(keys @ Wr, axis=-1)      # [B, LK]

    NQP, NKP, qoffs, koffs, LQG, NKG = _plan(aq, ak, M)

    key = (tuple(NQP), tuple(NKP), LQG, NKG, D, str(MMDT))
    if key not in _CACHE:
        _CACHE[key] = _build_program(NQP, NKP, qoffs, koffs, LQG, NKG, D)
    nc = _CACHE[key]

    # ---- gather + pad, build per-batch inputs ----
    perm_q = []   # original token ids, per batch, in gathered order
    slot_q = []   # gathered positions of those tokens
    XQTs, XKTs, XVTs, MQs, MKs = [], [], [], [], []
    for b in range(B):
        xq = np.zeros((LQG, D), np.float32)
        xk = np.zeros((NKG, D), np.float32)
        xv = np.zeros((NKG, D), np.float32)
        mqa = np.zeros((9, LQG), np.float32)
        mka = np.zeros((9, NKG), np.float32)
        mka[8, :] = SQRT_BIG
        pq, sq = [], []
        for c in range(M):
            tq = np.nonzero(aq[b] == c)[0]
            tk = np.nonzero(ak[b] == c)[0]
            xq[qoffs[c]:qoffs[c] + len(tq)] = queries[b, tq]
            xk[koffs[c]:koffs[c] + len(tk)] = keys[b, tk]
            xv[koffs[c]:koffs[c] + len(tk)] = values[b, tk]
            mqa[c, qoffs[c]:qoffs[c] + len(tq)] = SQRT_BIG
            mqa[8, qoffs[c]:qoffs[c] + len(tq)] = -SQRT_BIG
            mka[c, koffs[c]:koffs[c] + len(tk)] = SQRT_BIG
            pq.append(tq)
            sq.append(np.arange(qoffs[c], qoffs[c] + len(tq)))
        perm_q.append(np.concatenate(pq))
        slot_q.append(np.concatenate(sq))
        XQTs.append(np.ascontiguousarray(xq.T).astype(npdt))
        XKTs.append(np.ascontiguousarray(xk.T).astype(npdt))
        XVTs.append(np.ascontiguousarray(xv.T).astype(npdt))
        MQs.append(mqa.astype(npdt))
        MKs.append(mka.astype(npdt))

    in_maps = []
    for core in range(N_CORES):
        b, hg = core // HPC, core % HPC
        cols = slice(hg * HPC * DH, (hg + 1) * HPC * DH)
        in_maps.append({
            "XQT": XQTs[b], "XKT": XKTs[b], "XVT": XVTs[b],
            "WQ": np.ascontiguousarray(Wq[:, cols] * scale).astype(npdt),
            "WK": np.ascontiguousarray(Wk[:, cols]).astype(npdt),
            "WV": np.ascontiguousarray(Wv[:, cols]).astype(npdt),
            "WO": np.ascontiguousarray(Wo[cols, :]).astype(npdt),
            "MQ": MQs[b], "MK": MKs[b],
        })

    res = run_bass_kernel_spmd(nc, in_maps, list(range(N_CORES)), trace=trace)

    out = np.zeros((B, LQ, D), np.float32)
    for b in range(B):
        acc = res.results[b * HPC]["OUT"].copy()
        for hg in range(1, HPC):
            acc += res.results[b * HPC + hg]["OUT"]
        out[b, perm_q[b]] = acc[slot_q[b]]
    return out, res


def kernel(**inputs):
    out, _ = run(inputs)
    return out



# revision 5
# speedup vs baseline: 1.5375x; 1.5375x over previous
"""MoE clustered attention kernel for Trainium2 (8 NeuronCores), v2.

Problem: B=2, LQ=LK=2048, D=1024, H=16 heads (DH=64), M=8 clusters.
Each query/key token is routed (argmax of X @ Wr) to one of 8 clusters;
attention is only computed within a cluster (block-sparse attention).

Host: fp32 router argmax (matches the jax reference; top-2 logit gaps
for these inputs are far above rounding noise), gather tokens by
cluster, zero-pad each cluster to a common cross-batch geometry
(queries to multiples of 4, keys to multiples of 128), pre-transpose
X to [D, L] bf16.  Un-permute + sum the 4 head-group partials per
batch afterwards.

Device (per core = batch * 4 + head_group, 4 heads each), designed
around a gapless TensorE stream (TRN2 PE p-state ramping: any stall
drops the PE from 2.4 GHz to 1.2 GHz for the next 3 us):
  * head-pair packed projections: qT2/kT2 [128, L] tiles hold two
    heads' [64, L] blocks; n=512 matmuls, one [128, 512] PSUM
    evacuation per pair-tile alternating Vector/Scalar engines,
  * no mask rows anywhere: padded key columns of XKT are zero, so
    their scores are 0 and exp = 1; vA's mask block (below) zeroes
    their contribution to both the context and the denominator;
    padded q columns produce garbage that the host discards,
  * vA [128 keys, chunk, head, 128]: cols 0:64 a per-key 0/1 mask
    replicated 64x (memset 1.0 then one per-chunk tensor_scalar
    multiply by MSK), cols 64:128 projected values.  The ctx matmul
    (m=128) then yields rows 0:63 = 64 broadcast copies of the
    softmax denominator (at partition base 0, which the custom-DVE
    reciprocal requires) and rows 64:127 = unnormalized context, so
    normalization is reciprocal_approx_fast([64, nq]) + one
    tensor_tensor multiply straight out of PSUM -- no DMA staging,
    no gpsimd broadcast, no ACT table switches,
  * V-projection and attention interleaved in emission so ScalarE's
    exp stream hides under the TensorE window; per-block normalize
    emitted deferred-by-one so it never heads the DVE queue early,
  * exp supertiles: scores land in 2-bank [128, 1024] PSUM tiles,
    one strided ACTIVATE per pair of 128-key chunks (exp table
    preloaded by a dummy exp during Q-proj),
  * output projection n=512, bf16 output (host upcasts and sums).
PSUM: 2 proj + 4 scores + 2 ctx banks = 8.
"""

import numpy as np
import ml_dtypes

import concourse.bacc as bacc
import concourse.tile as tile
import concourse.mybir as mybir
from concourse.bass_utils import run_bass_kernel_spmd

F32 = mybir.dt.float32
BF16 = mybir.dt.bfloat16
EXP = mybir.ActivationFunctionType.Exp
MULT = mybir.AluOpType.mult

H = 16            # total heads
HPC = 4           # heads per core
N_CORES = 8
D = 1024
ND = D // 128


def _ceil_to(x, m):
    return (x + m - 1) // m * m


def _plan(aq, ak, M):
    """Common (cross-batch) padded cluster geometry."""
    B = aq.shape[0]
    nq = np.array([[int((aq[b] == c).sum()) for c in range(M)] for b in range(B)])
    nk = np.array([[int((ak[b] == c).sum()) for c in range(M)] for b in range(B)])
    NQP = [_ceil_to(int(nq[:, c].max()), 4) for c in range(M)]
    NKP = [_ceil_to(max(1, int(nk[:, c].max())), 128) for c in range(M)]
    qoff = np.concatenate([[0], np.cumsum(NQP)])
    koff = np.concatenate([[0], np.cumsum(NKP)])
    return NQP, NKP, qoff[:-1].tolist(), koff[:-1].tolist(), int(qoff[-1]), int(koff[-1])


def _build_program(NQP, NKP, qoffs, koffs, LQG, NKG):
    nc = bacc.Bacc("TRN2", target_bir_lowering=False, debug=False)
    XQT = nc.dram_tensor("XQT", [D, LQG], BF16, kind="ExternalInput").ap()
    XKT = nc.dram_tensor("XKT", [D, NKG], BF16, kind="ExternalInput").ap()
    XVT = nc.dram_tensor("XVT", [D, NKG], BF16, kind="ExternalInput").ap()
    WQ = nc.dram_tensor("WQ", [D, 256], BF16, kind="ExternalInput").ap()
    WK = nc.dram_tensor("WK", [D, 256], BF16, kind="ExternalInput").ap()
    WV = nc.dram_tensor("WV", [D, 256], BF16, kind="ExternalInput").ap()
    WO = nc.dram_tensor("WO", [256, D], BF16, kind="ExternalInput").ap()
    MSK = nc.dram_tensor("MSK", [128, NKG // 128], F32, kind="ExternalInput").ap()
    OUT = nc.dram_tensor("OUT", [LQG, D], BF16, kind="ExternalOutput").ap()

    NVC = NKG // 128
    M = len(NQP)
    # attention blocks: (cluster, qo, nq) with nq <= 512
    blocks = []
    for c in range(M):
        for qs in range(0, NQP[c], 512):
            blocks.append((c, qoffs[c] + qs, min(512, NQP[c] - qs)))

    with tile.TileContext(nc) as tc:
        with (
            tc.tile_pool(name="weights", bufs=1) as wpool,
            tc.tile_pool(name="big", bufs=1) as bigpool,
            tc.tile_pool(name="xin", bufs=3) as xpool,
            tc.tile_pool(name="es", bufs=4) as espool,
            tc.tile_pool(name="rbt", bufs=3) as rbtpool,
            tc.tile_pool(name="psP", bufs=2, space="PSUM") as psP,
            tc.tile_pool(name="psB", bufs=2, space="PSUM") as psB,
            tc.tile_pool(name="psC", bufs=2, space="PSUM") as psC,
        ):
            wq = wpool.tile([128, ND, 256], BF16, tag="wq")
            wk = wpool.tile([128, ND, 256], BF16, tag="wk")
            wv = wpool.tile([128, ND, 256], BF16, tag="wv")
            wo = wpool.tile([128, 2, 1024], BF16, tag="wo")
            msk = wpool.tile([128, NVC], F32, tag="msk")
            dummy = wpool.tile([1, 8], F32, tag="dummy")

            qT2 = [bigpool.tile([128, LQG], BF16, tag=f"qT{p}", name=f"qT{p}")
                   for p in range(2)]
            kT2 = [bigpool.tile([128, NKG], BF16, tag=f"kT{p}", name=f"kT{p}")
                   for p in range(2)]
            vA = bigpool.tile([128, NVC, HPC, 128], BF16, tag="vA")
            ctxT2 = [bigpool.tile([128, LQG], BF16, tag=f"cT{p}", name=f"cT{p}")
                     for p in range(2)]

            nc.sync.dma_start(wq[:], WQ.rearrange("(n p) m -> p n m", p=128))
            nc.sync.dma_start(wk[:], WK.rearrange("(n p) m -> p n m", p=128))
            nc.sync.dma_start(wv[:], WV.rearrange("(n p) m -> p n m", p=128))
            nc.sync.dma_start(wo[:], WO.rearrange("(n p) m -> p n m", p=128))
            nc.sync.dma_start(msk[:], MSK)

            # preload the exp ACT table off the critical path
            nc.vector.memset(dummy[:], 0.0)
            nc.scalar.activation(dummy[:], dummy[:], EXP)

            # vA mask block: memset 1.0, then zero padded keys per chunk
            nc.vector.memset(vA[:, :, :, 0:64], 1.0)
            for kc in range(NVC):
                nc.vector.tensor_scalar_mul(
                    vA[:, kc, :, 0:64], vA[:, kc, :, 0:64], msk[:, kc:kc + 1])

            # ---- Q/K projections (transposed, head-pair packed) ----
            def proj_T(xdram, L, wtile, dest, evac_flip):
                for off in range(0, L, 512):
                    w = min(512, L - off)
                    xt = xpool.tile([128, ND, 512], BF16, tag="xt")
                    nc.sync.dma_start(
                        xt[:, :, :w],
                        xdram.rearrange("(n p) m -> p n m", p=128)[:, :, off:off + w])
                    for pair in range(2):
                        ps = psP.tile([128, 512], F32, tag="psp")
                        for d in range(ND):
                            nc.tensor.matmul(
                                ps[:, :w],
                                wtile[:, d, pair * 128:(pair + 1) * 128],
                                xt[:, d, :w],
                                start=(d == 0), stop=(d == ND - 1))
                        if evac_flip[0]:
                            nc.vector.tensor_copy(dest[pair][:, off:off + w], ps[:, :w])
                        else:
                            nc.scalar.copy(dest[pair][:, off:off + w], ps[:, :w])
                        evac_flip[0] = not evac_flip[0]

            flip = [True]
            proj_T(XQT, LQG, wq, qT2, flip)
            proj_T(XKT, NKG, wk, kT2, flip)

            # ---- V projection + attention, interleaved emission ----
            def emit_vtile(t):
                lo = t * 512
                w = min(512, NKG - lo)
                xt = xpool.tile([128, ND, 512], BF16, tag="xt")
                nc.sync.dma_start(
                    xt[:, :, :w],
                    XVT.rearrange("(n p) m -> p n m", p=128)[:, :, lo:lo + w])
                for sub in range(w // 128):
                    kc = t * 4 + sub
                    ps = psP.tile([128, 256], F32, tag="psp")
                    for d in range(ND):
                        nc.tensor.matmul(ps[:],
                                         xt[:, d, sub * 128:(sub + 1) * 128],
                                         wv[:, d, :],
                                         start=(d == 0), stop=(d == ND - 1))
                    nc.vector.tensor_copy(
                        vA[:, kc, :, 64:128],
                        ps[:].rearrange("p (h e) -> p h e", h=HPC))

            pending = None  # (pair, rb, qo, nq, ps_c)

            def emit_norm(p):
                pair, rb, qo, nq, ps_c = p
                rbt = rbtpool.tile([64, 512], F32, tag="rbt")
                nc.vector.reciprocal_approx_fast(rbt[:, :nq], ps_c[0:64, :nq])
                nc.vector.tensor_tensor(ctxT2[pair][rb:rb + 64, qo:qo + nq],
                                        ps_c[64:128, :nq], rbt[:, :nq], MULT)

            nvt = (NKG + 511) // 512
            vt_emitted = 0
            for c, qo, nq in blocks:
                kc0 = koffs[c] // 128
                nkc = NKP[c] // 128
                while vt_emitted * 4 < kc0 + nkc and vt_emitted < nvt:
                    emit_vtile(vt_emitted)
                    vt_emitted += 1
                for h in range(HPC):
                    pair, rb = h // 2, (h % 2) * 64
                    es_list = []
                    first = True
                    for ki in range(0, nkc, 2):
                        nk2 = min(2, nkc - ki)
                        ps_s = psB.tile([128, 1024], F32, tag="ps_s")
                        e = espool.tile([128, 1024], BF16, tag="e")
                        for kj in range(nk2):
                            ko = koffs[c] + (ki + kj) * 128
                            nc.tensor.matmul(
                                ps_s[:, kj * 512: kj * 512 + nq],
                                kT2[pair][rb:rb + 64, ko:ko + 128],
                                qT2[pair][rb:rb + 64, qo:qo + nq],
                                start=True, stop=True)
                            es_list.append((e, kj * 512))
                        if first and pending is not None:
                            emit_norm(pending)
                            pending = None
                        first = False
                        pv = ps_s[:].rearrange("p (b n) -> p b n", b=2)[:, 0:nk2, 0:nq]
                        ev = e[:].rearrange("p (b n) -> p b n", b=2)[:, 0:nk2, 0:nq]
                        nc.scalar.activation(ev, pv, EXP)
                    ps_c = psC.tile([128, 512], F32, tag="ps_c")
                    for ki in range(nkc):
                        e, ecol = es_list[ki]
                        nc.tensor.matmul(ps_c[:, :nq],
                                         vA[:, kc0 + ki, h, :],
                                         e[:, ecol:ecol + nq],
                                         start=(ki == 0), stop=(ki == nkc - 1))
                    pending = (pair, rb, qo, nq, ps_c)
            emit_norm(pending)

        # ---- output projection ----
        with tc.tile_pool(name="ob", bufs=3) as obpool, \
             tc.tile_pool(name="psO", bufs=2, space="PSUM") as psO:
            oflip = True
            for mi in range((LQG + 127) // 128):
                mw = min(128, LQG - mi * 128)
                ob = obpool.tile([128, 1024], BF16, tag="ob")
                for n2 in range(2):
                    ps_o = psO.tile([128, 512], F32, tag="ps_o")
                    for pair in range(2):
                        nc.tensor.matmul(
                            ps_o[:mw, :],
                            ctxT2[pair][:, mi * 128: mi * 128 + mw],
                            wo[:, pair, n2 * 512:(n2 + 1) * 512],
                            start=(pair == 0), stop=(pair == 1))
                    if oflip:
                        nc.vector.tensor_copy(ob[:mw, n2 * 512:(n2 + 1) * 512],
                                              ps_o[:mw, :])
                    else:
                        nc.scalar.copy(ob[:mw, n2 * 512:(n2 + 1) * 512],
                                       ps_o[:mw, :])
                    oflip = not oflip
                nc.sync.dma_start(OUT[mi * 128: mi * 128 + mw, :], ob[:mw, :])

    nc.compile()
    return nc


_CACHE = {}


def run(inputs, trace=False):
    queries = np.asarray(inputs["queries"], np.float32)
    keys = np.asarray(inputs["keys"], np.float32)
    values = np.asarray(inputs["values"], np.float32)
    Wq = np.asarray(inputs["Wq"], np.float32)
    Wk = np.asarray(inputs["Wk"], np.float32)
    Wv = np.asarray(inputs["Wv"], np.float32)
    Wo = np.asarray(inputs["Wo"], np.float32)
    Wr = np.asarray(inputs["Wr"], np.float32)

    B, LQ, D_ = queries.shape
    M = Wr.shape[1]
    DH = D_ // H
    scale = np.float32(1.0 / np.sqrt(DH))
    npdt = ml_dtypes.bfloat16

    aq = np.argmax(queries @ Wr, axis=-1)   # [B, LQ]
    ak = np.argmax(keys @ Wr, axis=-1)      # [B, LK]

    NQP, NKP, qoffs, koffs, LQG, NKG = _plan(aq, ak, M)
    NVC = NKG // 128

    key = (tuple(NQP), tuple(NKP), LQG, NKG)
    if key not in _CACHE:
        _CACHE[key] = _build_program(NQP, NKP, qoffs, koffs, LQG, NKG)
    nc = _CACHE[key]

    # ---- gather + zero-pad, build per-batch inputs ----
    perm_q, slot_q = [], []
    XQTs, XKTs, XVTs, MSKs = [], [], [], []
    for b in range(B):
        xq = np.zeros((LQG, D_), np.float32)
        xk = np.zeros((NKG, D_), np.float32)
        xv = np.zeros((NKG, D_), np.float32)
        mska = np.zeros((NVC * 128,), np.float32)
        pq, sq = [], []
        for c in range(M):
            tq = np.nonzero(aq[b] == c)[0]
            tk = np.nonzero(ak[b] == c)[0]
            xq[qoffs[c]:qoffs[c] + len(tq)] = queries[b, tq]
            xk[koffs[c]:koffs[c] + len(tk)] = keys[b, tk]
            xv[koffs[c]:koffs[c] + len(tk)] = values[b, tk]
            mska[koffs[c]:koffs[c] + len(tk)] = 1.0
            pq.append(tq)
            sq.append(np.arange(qoffs[c], qoffs[c] + len(tq)))
        perm_q.append(np.concatenate(pq))
        slot_q.append(np.concatenate(sq))
        XQTs.append(np.ascontiguousarray(xq.T).astype(npdt))
        XKTs.append(np.ascontiguousarray(xk.T).astype(npdt))
        XVTs.append(np.ascontiguousarray(xv.T).astype(npdt))
        # msk[p, c] = real(key at chunk c, partition p)
        MSKs.append(np.ascontiguousarray(
            mska.reshape(NVC, 128).T))

    in_maps = []
    for core in range(N_CORES):
        b, hg = core // HPC, core % HPC
        cols = slice(hg * HPC * DH, (hg + 1) * HPC * DH)
        in_maps.append({
            "XQT": XQTs[b], "XKT": XKTs[b], "XVT": XVTs[b],
            "WQ": np.ascontiguousarray(Wq[:, cols] * scale).astype(npdt),
            "WK": np.ascontiguousarray(Wk[:, cols]).astype(npdt),
            "WV": np.ascontiguousarray(Wv[:, cols]).astype(npdt),
            "WO": np.ascontiguousarray(Wo[cols, :]).astype(npdt),
            "MSK": MSKs[b],
        })

    res = run_bass_kernel_spmd(nc, in_maps, list(range(N_CORES)), trace=trace)

    out = np.zeros((B, LQ, D_), np.float32)
    for b in range(B):
        acc = res.results[b * HPC]["OUT"].astype(np.float32)
        for hg in range(1, HPC):
            acc += res.results[b * HPC + hg]["OUT"].astype(np.float32)
        out[b, perm_q[b]] = acc[slot_q[b]]
    return out, res


def kernel(**inputs):
    out, _ = run(inputs)
    return out


# revision 7
# speedup vs baseline: 1.7027x; 1.1074x over previous
"""MoE clustered attention kernel for Trainium2 (8 NeuronCores), v2.

Problem: B=2, LQ=LK=2048, D=1024, H=16 heads (DH=64), M=8 clusters.
Each query/key token is routed (argmax of X @ Wr) to one of 8 clusters;
attention is only computed within a cluster (block-sparse attention).

Host: fp32 router argmax (matches the jax reference; top-2 logit gaps
for these inputs are far above rounding noise), gather tokens by
cluster, zero-pad each cluster to a common cross-batch geometry
(queries to multiples of 4, keys to multiples of 128), pre-transpose
X to [D, L] bf16.  Un-permute + sum the 4 head-group partials per
batch afterwards.

Device (per core = batch * 4 + head_group, 4 heads each), designed
around a gapless TensorE stream (TRN2 PE p-state ramping: any stall
drops the PE from 2.4 GHz to 1.2 GHz for the next 3 us):
  * head-pair packed projections: qT2/kT2 [128, L] tiles hold two
    heads' [64, L] blocks; n=512 matmuls, one [128, 512] PSUM
    evacuation per pair-tile alternating Vector/Scalar engines,
  * no mask rows anywhere: padded key columns of XKT are zero, so
    their scores are 0 and exp = 1; vA's mask block (below) zeroes
    their contribution to both the context and the denominator;
    padded q columns produce garbage that the host discards,
  * vA [128 keys, chunk, head, 128]: cols 0:64 a per-key 0/1 mask
    replicated 64x (memset 1.0 then one per-chunk tensor_scalar
    multiply by MSK), cols 64:128 projected values.  The ctx matmul
    (m=128) then yields rows 0:63 = 64 broadcast copies of the
    softmax denominator (at partition base 0, which the custom-DVE
    reciprocal requires) and rows 64:127 = unnormalized context, so
    normalization is reciprocal_approx_fast([64, nq]) + one
    tensor_tensor multiply straight out of PSUM -- no DMA staging,
    no gpsimd broadcast, no ACT table switches,
  * V-projection and attention interleaved in emission so ScalarE's
    exp stream hides under the TensorE window; per-block normalize
    emitted deferred-by-one so it never heads the DVE queue early,
  * exp supertiles: scores land in 2-bank [128, 1024] PSUM tiles,
    one strided ACTIVATE per pair of 128-key chunks (exp table
    preloaded by a dummy exp during Q-proj),
  * output projection n=512, bf16 output (host upcasts and sums).
PSUM: 2 proj + 4 scores + 2 ctx banks = 8.
"""

import numpy as np
import ml_dtypes

import concourse.bacc as bacc
import concourse.tile as tile
import concourse.mybir as mybir
from concourse.bass_utils import run_bass_kernel_spmd

F32 = mybir.dt.float32
BF16 = mybir.dt.bfloat16
EXP = mybir.ActivationFunctionType.Exp
MULT = mybir.AluOpType.mult

H = 16            # total heads
HPC = 4           # heads per core
N_CORES = 8
D = 1024
ND = D // 128


def _ceil_to(x, m):
    return (x + m - 1) // m * m


def _plan(aq, ak, M):
    """Common (cross-batch) padded cluster geometry."""
    B = aq.shape[0]
    nq = np.array([[int((aq[b] == c).sum()) for c in range(M)] for b in range(B)])
    nk = np.array([[int((ak[b] == c).sum()) for c in range(M)] for b in range(B)])
    NQP = [_ceil_to(int(nq[:, c].max()), 4) for c in range(M)]
    NKP = [_ceil_to(max(1, int(nk[:, c].max())), 128) for c in range(M)]
    qoff = np.concatenate([[0], np.cumsum(NQP)])
    koff = np.concatenate([[0], np.cumsum(NKP)])
    return NQP, NKP, qoff[:-1].tolist(), koff[:-1].tolist(), int(qoff[-1]), int(koff[-1])


def _build_program(NQP, NKP, qoffs, koffs, LQG, NKG):
    nc = bacc.Bacc("TRN2", target_bir_lowering=False, debug=False)
    XQT = nc.dram_tensor("XQT", [D, LQG], BF16, kind="ExternalInput").ap()
    XKT = nc.dram_tensor("XKT", [D, NKG], BF16, kind="ExternalInput").ap()
    XVT = nc.dram_tensor("XVT", [D, NKG], BF16, kind="ExternalInput").ap()
    WQ = nc.dram_tensor("WQ", [D, 256], BF16, kind="ExternalInput").ap()
    WK = nc.dram_tensor("WK", [D, 256], BF16, kind="ExternalInput").ap()
    WV = nc.dram_tensor("WV", [D, 256], BF16, kind="ExternalInput").ap()
    WO = nc.dram_tensor("WO", [256, D], BF16, kind="ExternalInput").ap()
    MSK = nc.dram_tensor("MSK", [128, NKG // 128], F32, kind="ExternalInput").ap()
    OUT = nc.dram_tensor("OUT", [LQG, D], BF16, kind="ExternalOutput").ap()

    NVC = NKG // 128
    M = len(NQP)
    # attention blocks: (cluster, qo, nq) with nq <= 512
    blocks = []
    for c in range(M):
        for qs in range(0, NQP[c], 512):
            blocks.append((c, qoffs[c] + qs, min(512, NQP[c] - qs)))

    with tile.TileContext(nc) as tc:
        with (
            tc.tile_pool(name="weights", bufs=1) as wpool,
            tc.tile_pool(name="big", bufs=1) as bigpool,
            tc.tile_pool(name="xin", bufs=5) as xpool,
            tc.tile_pool(name="es", bufs=4) as espool,
            tc.tile_pool(name="rbt", bufs=3) as rbtpool,
            tc.tile_pool(name="psP", bufs=2, space="PSUM") as psP,
            tc.tile_pool(name="psB", bufs=2, space="PSUM") as psB,
            tc.tile_pool(name="psC", bufs=2, space="PSUM") as psC,
        ):
            wq = wpool.tile([128, ND, 256], BF16, tag="wq")
            wk = wpool.tile([128, ND, 256], BF16, tag="wk")
            wv = wpool.tile([128, ND, 256], BF16, tag="wv")
            wo = wpool.tile([128, 2, 1024], BF16, tag="wo")
            msk = wpool.tile([128, NVC], F32, tag="msk")
            dummy = wpool.tile([1, 8], F32, tag="dummy")

            qT2 = [bigpool.tile([128, LQG], BF16, tag=f"qT{p}", name=f"qT{p}")
                   for p in range(2)]
            kT2 = [bigpool.tile([128, NKG], BF16, tag=f"kT{p}", name=f"kT{p}")
                   for p in range(2)]
            vA = bigpool.tile([128, NVC, HPC, 128], BF16, tag="vA")
            ctxT2 = [bigpool.tile([128, LQG], BF16, tag=f"cT{p}", name=f"cT{p}")
                     for p in range(2)]

            nc.sync.dma_start(wq[:], WQ.rearrange("(n p) m -> p n m", p=128))
            xt0 = xpool.tile([128, ND, 512], BF16, tag="xt", name="xt0")
            nc.sync.dma_start(xt0[:],
                              XQT.rearrange("(n p) m -> p n m", p=128)[:, :, 0:512])
            nc.sync.dma_start(wk[:], WK.rearrange("(n p) m -> p n m", p=128))
            nc.sync.dma_start(wv[:], WV.rearrange("(n p) m -> p n m", p=128))
            nc.sync.dma_start(wo[:], WO.rearrange("(n p) m -> p n m", p=128))
            nc.sync.dma_start(msk[:], MSK)

            # preload the exp ACT table off the critical path
            nc.vector.memset(dummy[:], 0.0)
            nc.scalar.activation(dummy[:], dummy[:], EXP)

            # vA mask block: memset 1.0; padded keys zeroed per chunk in the
            # V loop (so the multiplies never head the DVE queue waiting on MSK)
            nc.vector.memset(vA[:, :, :, 0:64], 1.0)

            # ---- Q/K projections (transposed, head-pair packed) ----
            def proj_T(xdram, L, wtile, dest, evac_flip, first_tile=None):
                for off in range(0, L, 512):
                    w = min(512, L - off)
                    if off == 0 and first_tile is not None:
                        xt = first_tile
                    else:
                        xt = xpool.tile([128, ND, 512], BF16, tag="xt")
                        nc.sync.dma_start(
                            xt[:, :, :w],
                            xdram.rearrange("(n p) m -> p n m", p=128)[:, :, off:off + w])
                    for pair in range(2):
                        ps = psP.tile([128, 512], F32, tag="psp")
                        for d in range(ND):
                            nc.tensor.matmul(
                                ps[:, :w],
                                wtile[:, d, pair * 128:(pair + 1) * 128],
                                xt[:, d, :w],
                                start=(d == 0), stop=(d == ND - 1))
                        if evac_flip[0]:
                            nc.vector.tensor_copy(dest[pair][:, off:off + w], ps[:, :w])
                        else:
                            nc.scalar.copy(dest[pair][:, off:off + w], ps[:, :w])
                        evac_flip[0] = not evac_flip[0]

            flip = [True]
            proj_T(XQT, LQG, wq, qT2, flip, first_tile=xt0)
            proj_T(XKT, NKG, wk, kT2, flip)

            # ---- V projection + attention, interleaved emission ----
            def emit_vtile(t):
                lo = t * 512
                w = min(512, NKG - lo)
                xt = xpool.tile([128, ND, 512], BF16, tag="xt")
                nc.sync.dma_start(
                    xt[:, :, :w],
                    XVT.rearrange("(n p) m -> p n m", p=128)[:, :, lo:lo + w])
                for sub in range(w // 128):
                    kc = t * 4 + sub
                    ps = psP.tile([128, 256], F32, tag="psp")
                    for d in range(ND):
                        nc.tensor.matmul(ps[:],
                                         xt[:, d, sub * 128:(sub + 1) * 128],
                                         wv[:, d, :],
                                         start=(d == 0), stop=(d == ND - 1))
                    nc.vector.tensor_copy(
                        vA[:, kc, :, 64:128],
                        ps[:].rearrange("p (h e) -> p h e", h=HPC))
                    nc.vector.tensor_scalar_mul(
                        vA[:, kc, :, 0:64], vA[:, kc, :, 0:64], msk[:, kc:kc + 1])

            # software-pipelined emission: iteration i emits scores+exp of
            # unit i, ctx of unit i-1, normalize of unit i-2 -- so the Tensor
            # stream never waits on the exp latency or the normalize chain.
            def emit_scores(u):
                c, qo, nq, h = u["c"], u["qo"], u["nq"], u["h"]
                pair, rb = h // 2, (h % 2) * 64
                es_list = []
                for ki in range(0, u["nkc"], 2):
                    nk2 = min(2, u["nkc"] - ki)
                    ps_s = psB.tile([128, 1024], F32, tag="ps_s")
                    e = espool.tile([128, 1024], BF16, tag="e")
                    for kj in range(nk2):
                        ko = koffs[c] + (ki + kj) * 128
                        nc.tensor.matmul(
                            ps_s[:, kj * 512: kj * 512 + nq],
                            kT2[pair][rb:rb + 64, ko:ko + 128],
                            qT2[pair][rb:rb + 64, qo:qo + nq],
                            start=True, stop=True)
                        es_list.append((e, kj * 512))
                    pv = ps_s[:].rearrange("p (b n) -> p b n", b=2)[:, 0:nk2, 0:nq]
                    ev = e[:].rearrange("p (b n) -> p b n", b=2)[:, 0:nk2, 0:nq]
                    nc.scalar.activation(ev, pv, EXP)
                u["es"] = es_list

            def emit_ctx(u):
                nq, h = u["nq"], u["h"]
                ps_c = psC.tile([128, 512], F32, tag="ps_c")
                for ki in range(u["nkc"]):
                    e, ecol = u["es"][ki]
                    nc.tensor.matmul(ps_c[:, :nq],
                                     vA[:, u["kc0"] + ki, h, :],
                                     e[:, ecol:ecol + nq],
                                     start=(ki == 0), stop=(ki == u["nkc"] - 1))
                u["ps_c"] = ps_c

            def emit_norm(u):
                pair, rb = u["h"] // 2, (u["h"] % 2) * 64
                qo, nq, ps_c = u["qo"], u["nq"], u["ps_c"]
                rbt = rbtpool.tile([64, 512], F32, tag="rbt")
                nc.vector.reciprocal_approx_fast(rbt[:, :nq], ps_c[0:64, :nq])
                nc.vector.tensor_tensor(ctxT2[pair][rb:rb + 64, qo:qo + nq],
                                        ps_c[64:128, :nq], rbt[:, :nq], MULT)

            units = []
            for c, qo, nq in blocks:
                for h in range(HPC):
                    units.append({"c": c, "qo": qo, "nq": nq, "h": h,
                                  "kc0": koffs[c] // 128, "nkc": NKP[c] // 128})

            nvt = (NKG + 511) // 512
            vt_emitted = 0
            for i, u in enumerate(units):
                need = u["kc0"] + u["nkc"]
                while vt_emitted * 4 < need and vt_emitted < nvt:
                    emit_vtile(vt_emitted)
                    vt_emitted += 1
                emit_scores(u)
                if i >= 1:
                    emit_ctx(units[i - 1])
                if i >= 2:
                    emit_norm(units[i - 2])
            emit_ctx(units[-1])
            emit_norm(units[-2])
            emit_norm(units[-1])

        # ---- output projection ----
        NMI = (LQG + 127) // 128
        OUTR = OUT.rearrange("(n p) m -> p n m", p=128) if LQG % 128 == 0 else None
        with tc.tile_pool(name="ob", bufs=3) as obpool, \
             tc.tile_pool(name="psO", bufs=4, space="PSUM") as psO:
            oflip = True
            ob = None
            for mi in range(NMI):
                mw = min(128, LQG - mi * 128)
                if mi % 2 == 0:
                    ob = obpool.tile([128, 2, 1024], BF16, tag="ob")
                half = mi % 2
                for n2 in range(2):
                    ps_o = psO.tile([128, 512], F32, tag="ps_o")
                    for pair in range(2):
                        nc.tensor.matmul(
                            ps_o[:mw, :],
                            ctxT2[pair][:, mi * 128: mi * 128 + mw],
                            wo[:, pair, n2 * 512:(n2 + 1) * 512],
                            start=(pair == 0), stop=(pair == 1))
                    if oflip:
                        nc.vector.tensor_copy(ob[:mw, half, n2 * 512:(n2 + 1) * 512],
                                              ps_o[:mw, :])
                    else:
                        nc.scalar.copy(ob[:mw, half, n2 * 512:(n2 + 1) * 512],
                                       ps_o[:mw, :])
                    oflip = not oflip
                last = (mi == NMI - 1)
                if half == 1 or last:
                    m0 = (mi - half) * 128
                    rows = min(256 if half else 128, LQG - m0)
                    nh = half + 1
                    if OUTR is not None and rows == nh * 128:
                        nc.sync.dma_start(OUTR[:, mi - half: mi + 1, :],
                                          ob[:, 0:nh, :])
                    else:
                        for j in range(nh):
                            r = min(128, LQG - (mi - half + j) * 128)
                            nc.sync.dma_start(
                                OUT[(mi - half + j) * 128:(mi - half + j) * 128 + r, :],
                                ob[:r, j, :])

    nc.compile()
    return nc


_CACHE = {}


def run(inputs, trace=False):
    queries = np.asarray(inputs["queries"], np.float32)
    keys = np.asarray(inputs["keys"], np.float32)
    values = np.asarray(inputs["values"], np.float32)
    Wq = np.asarray(inputs["Wq"], np.float32)
    Wk = np.asarray(inputs["Wk"], np.float32)
    Wv = np.asarray(inputs["Wv"], np.float32)
    Wo = np.asarray(inputs["Wo"], np.float32)
    Wr = np.asarray(inputs["Wr"], np.float32)

    B, LQ, D_ = queries.shape
    M = Wr.shape[1]
    DH = D_ // H
    scale = np.float32(1.0 / np.sqrt(DH))
    npdt = ml_dtypes.bfloat16

    aq = np.argmax(queries @ Wr, axis=-1)   # [B, LQ]
    ak = np.argmax(keys @ Wr, axis=-1)      # [B, LK]

    NQP, NKP, qoffs, koffs, LQG, NKG = _plan(aq, ak, M)
    NVC = NKG // 128

    key = (tuple(NQP), tuple(NKP), LQG, NKG)
    if key not in _CACHE:
        _CACHE[key] = _build_program(NQP, NKP, qoffs, koffs, LQG, NKG)
    nc = _CACHE[key]

    # ---- gather + zero-pad, build per-batch inputs ----
    perm_q, slot_q = [], []
    XQTs, XKTs, XVTs, MSKs = [], [], [], []
    for b in range(B):
        xq = np.zeros((LQG, D_), np.float32)
        xk = np.zeros((NKG, D_), np.float32)
        xv = np.zeros((NKG, D_), np.float32)
        mska = np.zeros((NVC * 128,), np.float32)
        pq, sq = [], []
        for c in range(M):
            tq = np.nonzero(aq[b] == c)[0]
            tk = np.nonzero(ak[b] == c)[0]
            xq[qoffs[c]:qoffs[c] + len(tq)] = queries[b, tq]
            xk[koffs[c]:koffs[c] + len(tk)] = keys[b, tk]
            xv[koffs[c]:koffs[c] + len(tk)] = values[b, tk]
            mska[koffs[c]:koffs[c] + len(tk)] = 1.0
            pq.append(tq)
            sq.append(np.arange(qoffs[c], qoffs[c] + len(tq)))
        perm_q.append(np.concatenate(pq))
        slot_q.append(np.concatenate(sq))
        XQTs.append(np.ascontiguousarray(xq.T).astype(npdt))
        XKTs.append(np.ascontiguousarray(xk.T).astype(npdt))
        XVTs.append(np.ascontiguousarray(xv.T).astype(npdt))
        # msk[p, c] = real(key at chunk c, partition p)
        MSKs.append(np.ascontiguousarray(
            mska.reshape(NVC, 128).T))

    in_maps = []
    for core in range(N_CORES):
        b, hg = core // HPC, core % HPC
        cols = slice(hg * HPC * DH, (hg + 1) * HPC * DH)
        in_maps.append({
            "XQT": XQTs[b], "XKT": XKTs[b], "XVT": XVTs[b],
            "WQ": np.ascontiguousarray(Wq[:, cols] * scale).astype(npdt),
            "WK": np.ascontiguousarray(Wk[:, cols]).astype(npdt),
            "WV": np.ascontiguousarray(Wv[:, cols]).astype(npdt),
            "WO": np.ascontiguousarray(Wo[cols, :]).astype(npdt),
            "MSK": MSKs[b],
        })

    res = run_bass_kernel_spmd(nc, in_maps, list(range(N_CORES)), trace=trace)

    out = np.zeros((B, LQ, D_), np.float32)
    for b in range(B):
        acc = res.results[b * HPC]["OUT"].astype(np.float32)
        for hg in range(1, HPC):
            acc += res.results[b * HPC + hg]["OUT"].astype(np.float32)
        out[b, perm_q[b]] = acc[slot_q[b]]
    return out, res


def kernel(**inputs):
    out, _ = run(inputs)
    return out


# revision 9
# speedup vs baseline: 1.7057x; 1.0018x over previous
"""MoE clustered attention kernel for Trainium2 (8 NeuronCores), v2.

Problem: B=2, LQ=LK=2048, D=1024, H=16 heads (DH=64), M=8 clusters.
Each query/key token is routed (argmax of X @ Wr) to one of 8 clusters;
attention is only computed within a cluster (block-sparse attention).

Host: fp32 router argmax (matches the jax reference; top-2 logit gaps
for these inputs are far above rounding noise), gather tokens by
cluster, zero-pad each cluster to a common cross-batch geometry
(queries to multiples of 4, keys to multiples of 128), pre-transpose
X to [D, L] bf16.  Un-permute + sum the 4 head-group partials per
batch afterwards.

Device (per core = batch * 4 + head_group, 4 heads each), designed
around a gapless TensorE stream (TRN2 PE p-state ramping: any stall
drops the PE from 2.4 GHz to 1.2 GHz for the next 3 us):
  * head-pair packed projections: qT2/kT2 [128, L] tiles hold two
    heads' [64, L] blocks; n=512 matmuls, one [128, 512] PSUM
    evacuation per pair-tile alternating Vector/Scalar engines,
  * no mask rows anywhere: padded key columns of XKT are zero, so
    their scores are 0 and exp = 1; vA's mask block (below) zeroes
    their contribution to both the context and the denominator;
    padded q columns produce garbage that the host discards,
  * vA [128 keys, chunk, head, 128]: cols 0:64 a per-key 0/1 mask
    replicated 64x (memset 1.0 then one per-chunk tensor_scalar
    multiply by MSK), cols 64:128 projected values.  The ctx matmul
    (m=128) then yields rows 0:63 = 64 broadcast copies of the
    softmax denominator (at partition base 0, which the custom-DVE
    reciprocal requires) and rows 64:127 = unnormalized context, so
    normalization is reciprocal_approx_fast([64, nq]) + one
    tensor_tensor multiply straight out of PSUM -- no DMA staging,
    no gpsimd broadcast, no ACT table switches,
  * V-projection and attention interleaved in emission so ScalarE's
    exp stream hides under the TensorE window; per-block normalize
    emitted deferred-by-one so it never heads the DVE queue early,
  * exp supertiles: scores land in 2-bank [128, 1024] PSUM tiles,
    one strided ACTIVATE per pair of 128-key chunks (exp table
    preloaded by a dummy exp during Q-proj),
  * output projection n=512, bf16 output (host upcasts and sums).
PSUM: 2 proj + 4 scores + 2 ctx banks = 8.
"""

import numpy as np
import ml_dtypes

import concourse.bacc as bacc
import concourse.tile as tile
import concourse.mybir as mybir
from concourse.bass_utils import run_bass_kernel_spmd

F32 = mybir.dt.float32
BF16 = mybir.dt.bfloat16
EXP = mybir.ActivationFunctionType.Exp
MULT = mybir.AluOpType.mult

H = 16            # total heads
HPC = 4           # heads per core
N_CORES = 8
D = 1024
ND = D // 128


def _ceil_to(x, m):
    return (x + m - 1) // m * m


def _plan(aq, ak, M):
    """Common (cross-batch) padded cluster geometry."""
    B = aq.shape[0]
    nq = np.array([[int((aq[b] == c).sum()) for c in range(M)] for b in range(B)])
    nk = np.array([[int((ak[b] == c).sum()) for c in range(M)] for b in range(B)])
    NQP = [_ceil_to(int(nq[:, c].max()), 4) for c in range(M)]
    NKP = [_ceil_to(max(1, int(nk[:, c].max())), 128) for c in range(M)]
    qoff = np.concatenate([[0], np.cumsum(NQP)])
    koff = np.concatenate([[0], np.cumsum(NKP)])
    return NQP, NKP, qoff[:-1].tolist(), koff[:-1].tolist(), int(qoff[-1]), int(koff[-1])


def _build_program(NQP, NKP, qoffs, koffs, ckoffs, nkmax, LQG, NKG, KL):
    nc = bacc.Bacc("TRN2", target_bir_lowering=False, debug=False)
    XQT = nc.dram_tensor("XQT", [D, LQG], BF16, kind="ExternalInput").ap()
    XKT = nc.dram_tensor("XKT", [D, KL], BF16, kind="ExternalInput").ap()
    XVT = nc.dram_tensor("XVT", [D, NKG], BF16, kind="ExternalInput").ap()
    WQ = nc.dram_tensor("WQ", [D, 256], BF16, kind="ExternalInput").ap()
    WK = nc.dram_tensor("WK", [D, 256], BF16, kind="ExternalInput").ap()
    WV = nc.dram_tensor("WV", [D, 256], BF16, kind="ExternalInput").ap()
    WO = nc.dram_tensor("WO", [256, D], BF16, kind="ExternalInput").ap()
    MSK = nc.dram_tensor("MSK", [128, NKG // 128], F32, kind="ExternalInput").ap()
    OUT = nc.dram_tensor("OUT", [LQG, D], BF16, kind="ExternalOutput").ap()

    NVC = NKG // 128
    M = len(NQP)
    # attention blocks: (cluster, qo, nq) with nq <= 512
    blocks = []
    for c in range(M):
        for qs in range(0, NQP[c], 512):
            blocks.append((c, qoffs[c] + qs, min(512, NQP[c] - qs)))

    with tile.TileContext(nc) as tc:
        with (
            tc.tile_pool(name="weights", bufs=1) as wpool,
            tc.tile_pool(name="big", bufs=1) as bigpool,
            tc.tile_pool(name="xin", bufs=5) as xpool,
            tc.tile_pool(name="es", bufs=4) as espool,
            tc.tile_pool(name="rbt", bufs=3) as rbtpool,
            tc.tile_pool(name="psP", bufs=2, space="PSUM") as psP,
            tc.tile_pool(name="psB", bufs=2, space="PSUM") as psB,
            tc.tile_pool(name="psC", bufs=2, space="PSUM") as psC,
        ):
            wq = wpool.tile([128, ND, 256], BF16, tag="wq")
            wk = wpool.tile([128, ND, 256], BF16, tag="wk")
            wv = wpool.tile([128, ND, 256], BF16, tag="wv")
            wo = wpool.tile([128, 2, 1024], BF16, tag="wo")
            msk = wpool.tile([128, NVC], F32, tag="msk")
            dummy = wpool.tile([1, 8], F32, tag="dummy")

            qT2 = [bigpool.tile([128, LQG], BF16, tag=f"qT{p}", name=f"qT{p}")
                   for p in range(2)]
            kTc = [bigpool.tile([128, KL], BF16, tag=f"kc{p}", name=f"kc{p}")
                   for p in range(2)]
            kT2 = [bigpool.tile([128, NKG], BF16, tag=f"kT{p}", name=f"kT{p}")
                   for p in range(2)]
            vA = bigpool.tile([128, NVC, HPC, 128], BF16, tag="vA")
            ctxT2 = [bigpool.tile([128, LQG], BF16, tag=f"cT{p}", name=f"cT{p}")
                     for p in range(2)]

            nc.sync.dma_start(wq[:], WQ.rearrange("(n p) m -> p n m", p=128))
            xt0 = xpool.tile([128, ND, 512], BF16, tag="xt", name="xt0")
            XQR = XQT.rearrange("(n p) m -> p n m", p=128)
            nc.sync.dma_start(xt0[:, 0:4, :], XQR[:, 0:4, 0:512])
            nc.sync.dma_start(xt0[:, 4:8, :], XQR[:, 4:8, 0:512])
            nc.gpsimd.dma_start(wk[:], WK.rearrange("(n p) m -> p n m", p=128))
            nc.gpsimd.dma_start(wv[:], WV.rearrange("(n p) m -> p n m", p=128))
            nc.gpsimd.dma_start(wo[:], WO.rearrange("(n p) m -> p n m", p=128))
            nc.gpsimd.dma_start(msk[:], MSK)

            # preload the exp ACT table off the critical path
            nc.vector.memset(dummy[:], 0.0)
            nc.scalar.activation(dummy[:], dummy[:], EXP)

            # vA mask block: memset 1.0; padded keys zeroed per chunk in the
            # V loop (so the multiplies never head the DVE queue waiting on MSK)
            nc.vector.memset(vA[:, :, :, 0:64], 1.0)

            # ---- Q/K projections (transposed, head-pair packed) ----
            def proj_T(xdram, L, wtile, dest, evac_flip, first_tile=None):
                for off in range(0, L, 512):
                    w = min(512, L - off)
                    if off == 0 and first_tile is not None:
                        xt = first_tile
                    else:
                        xt = xpool.tile([128, ND, 512], BF16, tag="xt")
                        nc.sync.dma_start(
                            xt[:, :, :w],
                            xdram.rearrange("(n p) m -> p n m", p=128)[:, :, off:off + w])
                    for pair in range(2):
                        ps = psP.tile([128, 512], F32, tag="psp")
                        for d in range(ND):
                            nc.tensor.matmul(
                                ps[:, :w],
                                wtile[:, d, pair * 128:(pair + 1) * 128],
                                xt[:, d, :w],
                                start=(d == 0), stop=(d == ND - 1))
                        if evac_flip[0]:
                            nc.vector.tensor_copy(dest[pair][:, off:off + w], ps[:, :w])
                        else:
                            nc.scalar.copy(dest[pair][:, off:off + w], ps[:, :w])
                        evac_flip[0] = not evac_flip[0]

            flip = [True]
            proj_T(XQT, LQG, wq, qT2, flip, first_tile=xt0)
            # zero kT2 early (padded key cols must read 0 -> exp = 1, masked)
            nc.vector.memset(kT2[0][:], 0.0)
            nc.gpsimd.memset(kT2[1][:], 0.0)
            proj_T(XKT, KL, wk, kTc, flip)
            M_ = len(NQP)
            for c in range(M_):
                for pair in range(2):
                    w = nkmax[c]
                    if flip[0]:
                        nc.vector.tensor_copy(kT2[pair][:, koffs[c]:koffs[c] + w],
                                              kTc[pair][:, ckoffs[c]:ckoffs[c] + w])
                    else:
                        nc.scalar.copy(kT2[pair][:, koffs[c]:koffs[c] + w],
                                       kTc[pair][:, ckoffs[c]:ckoffs[c] + w])
                    flip[0] = not flip[0]

            # ---- V projection + attention, interleaved emission ----
            def emit_vtile(t):
                lo = t * 512
                w = min(512, NKG - lo)
                xt = xpool.tile([128, ND, 512], BF16, tag="xt")
                nc.sync.dma_start(
                    xt[:, :, :w],
                    XVT.rearrange("(n p) m -> p n m", p=128)[:, :, lo:lo + w])
                for sub in range(w // 128):
                    kc = t * 4 + sub
                    ps = psP.tile([128, 256], F32, tag="psp")
                    for d in range(ND):
                        nc.tensor.matmul(ps[:],
                                         xt[:, d, sub * 128:(sub + 1) * 128],
                                         wv[:, d, :],
                                         start=(d == 0), stop=(d == ND - 1))
                    nc.vector.tensor_copy(
                        vA[:, kc, :, 64:128],
                        ps[:].rearrange("p (h e) -> p h e", h=HPC))
                    nc.vector.tensor_scalar_mul(
                        vA[:, kc, :, 0:64], vA[:, kc, :, 0:64], msk[:, kc:kc + 1])

            # software-pipelined emission: iteration i emits scores+exp of
            # unit i, ctx of unit i-1, normalize of unit i-2 -- so the Tensor
            # stream never waits on the exp latency or the normalize chain.
            def emit_scores(u):
                c, qo, nq, h = u["c"], u["qo"], u["nq"], u["h"]
                pair, rb = h // 2, (h % 2) * 64
                es_list = []
                for ki in range(0, u["nkc"], 2):
                    nk2 = min(2, u["nkc"] - ki)
                    ps_s = psB.tile([128, 1024], F32, tag="ps_s")
                    e = espool.tile([128, 1024], BF16, tag="e")
                    for kj in range(nk2):
                        ko = koffs[c] + (ki + kj) * 128
                        nc.tensor.matmul(
                            ps_s[:, kj * 512: kj * 512 + nq],
                            kT2[pair][rb:rb + 64, ko:ko + 128],
                            qT2[pair][rb:rb + 64, qo:qo + nq],
                            start=True, stop=True)
                        es_list.append((e, kj * 512))
                    pv = ps_s[:].rearrange("p (b n) -> p b n", b=2)[:, 0:nk2, 0:nq]
                    ev = e[:].rearrange("p (b n) -> p b n", b=2)[:, 0:nk2, 0:nq]
                    nc.scalar.activation(ev, pv, EXP)
                u["es"] = es_list

            def emit_ctx(u):
                nq, h = u["nq"], u["h"]
                ps_c = psC.tile([128, 512], F32, tag="ps_c")
                for ki in range(u["nkc"]):
                    e, ecol = u["es"][ki]
                    nc.tensor.matmul(ps_c[:, :nq],
                                     vA[:, u["kc0"] + ki, h, :],
                                     e[:, ecol:ecol + nq],
                                     start=(ki == 0), stop=(ki == u["nkc"] - 1))
                u["ps_c"] = ps_c

            def emit_norm(u):
                pair, rb = u["h"] // 2, (u["h"] % 2) * 64
                qo, nq, ps_c = u["qo"], u["nq"], u["ps_c"]
                rbt = rbtpool.tile([64, 512], F32, tag="rbt")
                nc.vector.reciprocal_approx_fast(rbt[:, :nq], ps_c[0:64, :nq])
                nc.vector.tensor_tensor(ctxT2[pair][rb:rb + 64, qo:qo + nq],
                                        ps_c[64:128, :nq], rbt[:, :nq], MULT)

            units = []
            for c, qo, nq in blocks:
                for h in range(HPC):
                    units.append({"c": c, "qo": qo, "nq": nq, "h": h,
                                  "kc0": koffs[c] // 128, "nkc": NKP[c] // 128})

            nvt = (NKG + 511) // 512
            vt_emitted = 0
            for i, u in enumerate(units):
                need = u["kc0"] + u["nkc"]
                while vt_emitted * 4 < need and vt_emitted < nvt:
                    emit_vtile(vt_emitted)
                    vt_emitted += 1
                emit_scores(u)
                if i >= 1:
                    emit_ctx(units[i - 1])
                if i >= 2:
                    emit_norm(units[i - 2])
            emit_ctx(units[-1])
            emit_norm(units[-2])
            emit_norm(units[-1])

        # ---- output projection ----
        NMI = (LQG + 127) // 128
        OUTR = OUT.rearrange("(n p) m -> p n m", p=128) if LQG % 128 == 0 else None
        with tc.tile_pool(name="ob", bufs=3) as obpool, \
             tc.tile_pool(name="psO", bufs=4, space="PSUM") as psO:
            oflip = True
            ob = None
            for mi in range(NMI):
                mw = min(128, LQG - mi * 128)
                if mi % 2 == 0:
                    ob = obpool.tile([128, 2, 1024], BF16, tag="ob")
                half = mi % 2
                for n2 in range(2):
                    ps_o = psO.tile([128, 512], F32, tag="ps_o")
                    for pair in range(2):
                        nc.tensor.matmul(
                            ps_o[:mw, :],
                            ctxT2[pair][:, mi * 128: mi * 128 + mw],
                            wo[:, pair, n2 * 512:(n2 + 1) * 512],
                            start=(pair == 0), stop=(pair == 1))
                    if oflip:
                        nc.vector.tensor_copy(ob[:mw, half, n2 * 512:(n2 + 1) * 512],
                                              ps_o[:mw, :])
                    else:
                        nc.scalar.copy(ob[:mw, half, n2 * 512:(n2 + 1) * 512],
                                       ps_o[:mw, :])
                    oflip = not oflip
                last = (mi == NMI - 1)
                if half == 1 or last:
                    m0 = (mi - half) * 128
                    rows = min(256 if half else 128, LQG - m0)
                    nh = half + 1
                    if OUTR is not None and rows == nh * 128:
                        nc.gpsimd.dma_start(OUTR[:, mi - half: mi + 1, :],
                                          ob[:, 0:nh, :])
                    else:
                        for j in range(nh):
                            r = min(128, LQG - (mi - half + j) * 128)
                            nc.gpsimd.dma_start(
                                OUT[(mi - half + j) * 128:(mi - half + j) * 128 + r, :],
                                ob[:r, j, :])

    nc.compile()
    return nc


_CACHE = {}


def run(inputs, trace=False):
    queries = np.asarray(inputs["queries"], np.float32)
    keys = np.asarray(inputs["keys"], np.float32)
    values = np.asarray(inputs["values"], np.float32)
    Wq = np.asarray(inputs["Wq"], np.float32)
    Wk = np.asarray(inputs["Wk"], np.float32)
    Wv = np.asarray(inputs["Wv"], np.float32)
    Wo = np.asarray(inputs["Wo"], np.float32)
    Wr = np.asarray(inputs["Wr"], np.float32)

    B, LQ, D_ = queries.shape
    M = Wr.shape[1]
    DH = D_ // H
    scale = np.float32(1.0 / np.sqrt(DH))
    npdt = ml_dtypes.bfloat16

    aq = np.argmax(queries @ Wr, axis=-1)   # [B, LQ]
    ak = np.argmax(keys @ Wr, axis=-1)      # [B, LK]

    NQP, NKP, qoffs, koffs, LQG, NKG = _plan(aq, ak, M)
    NVC = NKG // 128
    nk = np.array([[int((ak[b] == c).sum()) for c in range(M)] for b in range(B)])
    nkmax = [int(nk[:, c].max()) for c in range(M)]
    ckoff = np.concatenate([[0], np.cumsum(nkmax)])
    ckoffs = ckoff[:-1].tolist()
    KL = _ceil_to(int(ckoff[-1]), 4)

    key = (tuple(NQP), tuple(NKP), LQG, NKG, KL, tuple(nkmax))
    if key not in _CACHE:
        _CACHE[key] = _build_program(NQP, NKP, qoffs, koffs, ckoffs, nkmax,
                                     LQG, NKG, KL)
    nc = _CACHE[key]

    # ---- gather + zero-pad, build per-batch inputs ----
    perm_q, slot_q = [], []
    XQTs, XKTs, XVTs, MSKs = [], [], [], []
    for b in range(B):
        xq = np.zeros((LQG, D_), np.float32)
        xk = np.zeros((KL, D_), np.float32)
        xv = np.zeros((NKG, D_), np.float32)
        mska = np.zeros((NVC * 128,), np.float32)
        pq, sq = [], []
        for c in range(M):
            tq = np.nonzero(aq[b] == c)[0]
            tk = np.nonzero(ak[b] == c)[0]
            xq[qoffs[c]:qoffs[c] + len(tq)] = queries[b, tq]
            xk[ckoffs[c]:ckoffs[c] + len(tk)] = keys[b, tk]
            xv[koffs[c]:koffs[c] + len(tk)] = values[b, tk]
            mska[koffs[c]:koffs[c] + len(tk)] = 1.0
            pq.append(tq)
            sq.append(np.arange(qoffs[c], qoffs[c] + len(tq)))
        perm_q.append(np.concatenate(pq))
        slot_q.append(np.concatenate(sq))
        XQTs.append(np.ascontiguousarray(xq.T).astype(npdt))
        XKTs.append(np.ascontiguousarray(xk.T).astype(npdt))
        XVTs.append(np.ascontiguousarray(xv.T).astype(npdt))
        # msk[p, c] = real(key at chunk c, partition p)
        MSKs.append(np.ascontiguousarray(
            mska.reshape(NVC, 128).T))

    in_maps = []
    for core in range(N_CORES):
        b, hg = core // HPC, core % HPC
        cols = slice(hg * HPC * DH, (hg + 1) * HPC * DH)
        in_maps.append({
            "XQT": XQTs[b], "XKT": XKTs[b], "XVT": XVTs[b],
            "WQ": np.ascontiguousarray(Wq[:, cols] * scale).astype(npdt),
            "WK": np.ascontiguousarray(Wk[:, cols]).astype(npdt),
            "WV": np.ascontiguousarray(Wv[:, cols]).astype(npdt),
            "WO": np.ascontiguousarray(Wo[cols, :]).astype(npdt),
            "MSK": MSKs[b],
        })

    res = run_bass_kernel_spmd(nc, in_maps, list(range(N_CORES)), trace=trace)

    out = np.zeros((B, LQ, D_), np.float32)
    for b in range(B):
        acc = res.results[b * HPC]["OUT"].astype(np.float32)
        for hg in range(1, HPC):
            acc += res.results[b * HPC + hg]["OUT"].astype(np.float32)
        out[b, perm_q[b]] = acc[slot_q[b]]
    return out, res


def kernel(**inputs):
    out, _ = run(inputs)
    return out
